# revision 14
# baseline (speedup 1.0000x reference)
"""Multi-head causal attention (B=8, T=1024, C=1024, H=16, hs=64) on 8 trn2 cores.

Data-parallel over batch: core b computes full attention for x[b].

Device algorithm (per core), all matmuls bf16 inputs / fp32 PSUM accum:
  - xT [C, T] resident in SBUF (host pre-transposed, bf16).
  - PE warmup: dummy matmuls at t=0 so the HAM clock-gate releases (K=8/8)
    before real work arrives.
  - v computed up-front for heads 0..13 (N=512/N=384 streams); heads 14,15
    deferred into pair 7's body to cover the final exp drain.
  - per head-pair (2 heads packed on partitions): qT, kT = W^T @ xT -> [128, T].
  - scores transposed: scT[s_tile, t] = kT_chunk^T @ qT (only causal blocks),
    exp fused on ScalarE (scale=1/8) with ONE instruction per (i, span)
    covering both heads via strided 3D APs; diagonal 128x128 block masked by
    a tril multiply on VectorE.
  - AV transposed: out[t, d] += es_chunk^T @ [v | 1] per (head, t-chunk,
    s-chunk): M=128 (full array), N=65; column 64 accumulates the softmax
    denominator, which lands as a per-partition scalar.
  - normalize: DVE reciprocal of col 64 + tensor_scalar multiply into an
    SBUF staging tile; one DMA per head writes out[T, 64h:64h+64] directly.
"""

import numpy as np
import ml_dtypes

import concourse.bass as bass
import concourse.mybir as mybir
from concourse import bacc
from concourse.tile import TileContext
from concourse.bass import ds, ts
from concourse.bass_utils import run_bass_kernel_spmd
from concourse.masks import make_upper_triangular

BF16 = mybir.dt.bfloat16
F32 = mybir.dt.float32

B, T, C, H, HS = 8, 1024, 1024, 16, 64
P = 128
CK = C // P       # 8 contraction chunks
TT = T // P       # 8 t tiles
PAIRS = H // 2    # 8 head pairs
HALF = 512
NV = HS + 1       # v cols + ones col (denominator)

_BUILT = None


def build_nc():
    nc = bacc.Bacc("TRN2", target_bir_lowering=False, debug=False)
    # [p, c, t] : xT[C, T] chunked; partition p, chunk c -> row 128c+p of xT
    xt = nc.dram_tensor("xt", [P, CK, T], BF16, kind="ExternalInput")
    # [proj(q,k), pair, p, c, f] : lhsT chunks, f = 2 heads x 64 stacked
    wqk = nc.dram_tensor("wqk", [2, PAIRS, P, CK, P], BF16, kind="ExternalInput")
    # [p, c, pair, f]
    wv = nc.dram_tensor("wv", [P, CK, PAIRS, P], BF16, kind="ExternalInput")
    # out[T, H*HS] directly -- no host transpose needed
    out = nc.dram_tensor("out", [T, H * HS], F32, kind="ExternalOutput")

    with TileContext(nc) as tc:
        with (
            tc.tile_pool(name="const", bufs=1) as constp,
            tc.tile_pool(name="wpool", bufs=8) as wpool,
            tc.tile_pool(name="qkpool", bufs=8) as qkp,
            tc.tile_pool(name="exppool", bufs=20) as expp,
            tc.tile_pool(name="stgpool", bufs=4) as stgp,
            tc.tile_pool(name="rcppool", bufs=8) as rcpp,
            tc.tile_pool(name="psA", bufs=2, space="PSUM") as psA,
            tc.tile_pool(name="psSc", bufs=2, space="PSUM") as psSc,
            tc.tile_pool(name="psV", bufs=2, space="PSUM") as psV,
        ):
            # ---- PE warmup: keep the array busy from t=0 so HAM goes
            # K=8/8 before the first real projection matmul. ----
            scratch = constp.tile([P, HALF], BF16)
            nc.gpsimd.memset(scratch[:, :], 0.0)
            for wi in range(12):
                pw = psA.tile([P, HALF], F32, tag="ps", name=f"warm{wi}")
                nc.tensor.matmul(pw[:, :], scratch[:, 0:P], scratch[:, :],
                                 start=True, stop=True)

            xt_sb = constp.tile([P, CK, T], BF16)
            # pair-group-major: [p, c, pg, 4*128] so the rhs slice is 2D
            wv_sb = constp.tile([P, CK, 2, 4 * P], BF16)
            # pair-0 Q/K weights first, then xt per chunk: the pair-0
            # projection starts as soon as chunk 0 lands. wv follows xt.
            wq0 = wpool.tile([P, CK, P], BF16, tag="w", name="wq0")
            nc.sync.dma_start(wq0[:, :, :], wqk[0, 0, :, :, :])
            wk0 = wpool.tile([P, CK, P], BF16, tag="w", name="wk0")
            nc.sync.dma_start(wk0[:, :, :], wqk[1, 0, :, :, :])
            for c in range(CK):
                nc.sync.dma_start(xt_sb[:, c, :], xt[:, c, :])
            for c in range(CK):
                nc.sync.dma_start(
                    wv_sb[:, c, :, :],
                    wv[:, c, :, :].rearrange("p (g r) f -> p g (r f)", g=2),
                )
            mask = constp.tile([P, P], BF16)
            make_upper_triangular(nc, mask, val=1.0, diag=True)
            # [s_p, head, s_tile, 64 v cols + 1 ones col]
            v_all = constp.tile([P, H, TT, NV], BF16)
            nc.gpsimd.memset(v_all[:, :, :, HS:NV], 1.0)

            # ---- attention per head pair, software-pipelined ----
            w_next = {}

            def dma_w(pair):
                wq_sb = wpool.tile([P, CK, P], BF16, tag="w", name=f"wq{pair}")
                nc.sync.dma_start(wq_sb[:, :, :], wqk[0, pair, :, :, :])
                wk_sb = wpool.tile([P, CK, P], BF16, tag="w", name=f"wk{pair}")
                nc.sync.dma_start(wk_sb[:, :, :], wqk[1, pair, :, :, :])
                w_next[pair] = (wq_sb, wk_sb)

            def qk_proj(pair):
                qT = qkp.tile([P, T], BF16, tag="qk", name=f"q{pair}")
                kT = qkp.tile([P, T], BF16, tag="qk", name=f"k{pair}")
                for wsb, dst in zip(w_next.pop(pair), (qT, kT)):
                    for g in range(2):
                        pp = psA.tile([P, HALF], F32, tag="ps",
                                      name=f"pp{pair}_{g}")
                        for c in range(CK):
                            nc.tensor.matmul(
                                pp[:, :],
                                wsb[:, c, :],
                                xt_sb[:, c, ds(HALF * g, HALF)],
                                start=(c == 0),
                                stop=(c == CK - 1),
                            )
                        nc.vector.tensor_copy(dst[:, ds(HALF * g, HALF)],
                                              pp[:, :])
                return qT, kT

            # AV for `pair`, software-pipelined one pair behind the scores:
            # running it in the NEXT pair's body breaks the serial
            # exp(p) -> AV(p) -> scores(p+1) -> exp(p+1) dependency cycle
            # that otherwise stretches the pair period to the ACT drain.
            def av_block(pair, es):
                # g-outer / head-inner: head hh's normalize (DVE) overlaps
                # the other head's matmul burst, so the psV slot handoff
                # never idles the PE (matters most for the last pair's tail).
                stgs = [stgp.tile([P, TT, HS], F32, tag="stg",
                                  name=f"stg{pair}_{hh}") for hh in range(2)]
                for g in range(2):
                    for hh in range(2):
                        h = 2 * pair + hh
                        av = psV.tile([P, 4, NV], F32, tag="av",
                                      name=f"av{hh}_{g}")
                        for jj in range(4):
                            j = 4 * g + jj
                            for i in range(j + 1):
                                nc.tensor.matmul(
                                    av[:, jj, :],
                                    es[i][:, ds(T * hh + P * j, P)],
                                    v_all[:, h, i, :],
                                    start=(i == 0),
                                    stop=(i == j),
                                )
                        rcp = rcpp.tile([P, 4, 1], F32, tag="rcp",
                                        name=f"rcp{hh}_{g}")
                        nc.vector.reciprocal(rcp[:, :, :], av[:, :, HS:NV])
                        in0b, in1b = bass.broadcast_tensor_aps(
                            av[:, :, 0:HS], rcp[:, :, :])
                        nc.vector.tensor_tensor(
                            stgs[hh][:, ds(4 * g, 4), :], in0b, in1b,
                            mybir.AluOpType.mult,
                        )
                for hh in range(2):
                    h = 2 * pair + hh
                    nc.sync.dma_start(
                        out.rearrange("(j p) c -> p j c", p=P)[:, :,
                                                              ds(HS * h, HS)],
                        stgs[hh][:, :, :],
                    )

            # ---- prologue: pair-0/1/2 projections interleaved with the v
            # projections. The v work is pure PE filler with no ACT cost, so
            # it runs while the scheduler hoists sc(0..2)'s exp onto the
            # otherwise-idle ScalarE; the pair loop is ACT-bound, so v must
            # NOT sit in a separate phase that starves the ACT. ----
            def v_group(h0, nh, pg, col0, name):
                """v for heads [h0, h0+nh): rhs = wv_sb[:, c, pg, col0:...]"""
                for j in range(TT):
                    pv = psA.tile([P, HALF], F32, tag="ps", name=f"{name}_{j}")
                    for c in range(CK):
                        nc.tensor.matmul(
                            pv[:, 0:HS * nh],
                            xt_sb[:, c, ts(j, P)],
                            wv_sb[:, c, pg, ds(col0, HS * nh)],
                            start=(c == 0),
                            stop=(c == CK - 1),
                        )
                    nc.vector.tensor_copy(
                        v_all[:, ds(h0, nh), j, 0:HS],
                        pv[:, 0:HS * nh].rearrange("p (g d) -> p g d", d=HS),
                    )

            w_next[0] = (wq0, wk0)
            dma_w(1)
            dma_w(2)
            dma_w(3)
            qk_ready = {0: qk_proj(0)}
            qk_ready[1] = qk_proj(1)
            qk_ready[2] = qk_proj(2)
            v_group(0, 4, 0, 0, "v01")
            v_group(4, 4, 0, 256, "v23")
            v_group(8, 8, 1, 0, "v47")

            es_prev = None
            for pair in range(PAIRS):
                qT, kT = qk_ready.pop(pair)

                # scores^T + exp for BOTH heads: two K=64 matmuls in row
                # groups (0,0)/(64,0) run concurrently; ONE merged ACT per
                # (i, span) covers both heads via stride-512 / stride-1024
                # 3D APs. es tile: [128, 2048], head w at col 1024*w.
                es = []
                for i in range(TT):
                    t0 = P * i
                    e = expp.tile([P, 2 * T], BF16, tag="exp", name=f"e{i}")
                    es.append(e)
                    e3 = e.rearrange("p (w t) -> p w t", w=2)
                    spans = [(t0, HALF), (HALF, T)] if t0 < HALF else [(t0, T)]
                    for a, b in spans:
                        sc = psSc.tile([P, T], F32, tag="sc", name=f"sc{i}_{a}")
                        for w in range(2):
                            po = HS * w
                            nc.tensor.matmul(
                                sc[:, ds(HALF * w, b - a)],
                                kT[ds(po, HS), ds(t0, P)],
                                qT[ds(po, HS), ds(a, b - a)],
                            )
                        nc.scalar.activation(
                            e3[:, :, a:b],
                            sc.rearrange("p (w t) -> p w t", w=2)[:, :, 0:b - a],
                            mybir.ActivationFunctionType.Exp,
                            scale=HS ** -0.5,
                        )
                    for w in range(2):
                        nc.vector.tensor_tensor(
                            e3[:, w, t0:t0 + P], e3[:, w, t0:t0 + P],
                            mask[:, :], mybir.AluOpType.mult,
                        )

                # pair p+3's projection runs here, covering the exp drain
                if pair + 3 < PAIRS:
                    qk_ready[pair + 3] = qk_proj(pair + 3)
                if pair + 4 < PAIRS:
                    dma_w(pair + 4)

                # AV transposed (one pair behind): out[t, d] += es^T @ [v|1].
                # M=128 (full), N=65; col 64 = softmax denominator.
                if es_prev is not None:
                    av_block(pair - 1, es_prev)
                es_prev = es
            av_block(PAIRS - 1, es_prev)
    nc.compile()
    return nc


def get_nc():
    global _BUILT
    if _BUILT is None:
        _BUILT = build_nc()
    return _BUILT


def prep_inputs(x, Wq, Wk, Wv):
    """Host-side shard + layout prep. Returns in_maps (one dict per core)."""
    x = np.asarray(x, dtype=np.float32)
    Wq = np.asarray(Wq, dtype=np.float32)
    Wk = np.asarray(Wk, dtype=np.float32)
    Wv = np.asarray(Wv, dtype=np.float32)
    bf = ml_dtypes.bfloat16

    # xT[b]: [C, T] -> [p, c, t] with row 128c+p
    xts = []
    for b in range(B):
        xT = np.ascontiguousarray(x[b].T)          # [C, T]
        xts.append(xT.reshape(CK, P, T).transpose(1, 0, 2).astype(bf))

    def pack_pairs(W):
        # [H, C, hs] -> [pair, C, 128] -> [pair, p, c, f]
        Wp = W.reshape(PAIRS, 2, C, HS).transpose(0, 2, 1, 3).reshape(PAIRS, C, P)
        return Wp.reshape(PAIRS, CK, P, P).transpose(0, 2, 1, 3)  # [pair, p, c, f]

    wq_p = pack_pairs(Wq)
    wk_p = pack_pairs(Wk)
    wqk_host = np.stack([wq_p, wk_p], axis=0).astype(bf)  # [2, pair, p, c, f]
    # wv: [p, c, pair, f]
    wv_host = np.ascontiguousarray(pack_pairs(Wv).transpose(1, 2, 0, 3)).astype(bf)

    return [
        {"xt": np.ascontiguousarray(xts[b]), "wqk": wqk_host, "wv": wv_host}
        for b in range(B)
    ]


def run_on_device(in_maps, **kwargs):
    nc = get_nc()
    return run_bass_kernel_spmd(nc, in_maps, list(range(B)), **kwargs)


def assemble(core_out):
    """Device already writes [T, H*HS]."""
    return np.asarray(core_out)


def kernel(x, Wq, Wk, Wv):
    in_maps = prep_inputs(x, Wq, Wk, Wv)
    res = run_on_device(in_maps)
    return np.stack([assemble(res.results[b]["out"]) for b in range(B)], axis=0)


# revision 16
# speedup vs baseline: 1.0329x; 1.0329x over previous
"""Multi-head causal attention (B=8, T=1024, C=1024, H=16, hs=64) on 8 trn2 cores.

Data-parallel over batch: core b computes full attention for x[b].

Device algorithm (per core), all matmuls bf16 inputs / fp32 PSUM accum:
  - xT [C, T] resident in SBUF (host pre-transposed, bf16).
  - PE warmup: dummy matmuls at t=0 so the HAM clock-gate releases (K=8/8)
    before real work arrives.
  - v computed up-front for heads 0..13 (N=512/N=384 streams); heads 14,15
    deferred into pair 7's body to cover the final exp drain.
  - per head-pair (2 heads packed on partitions): qT, kT = W^T @ xT -> [128, T].
  - scores transposed: scT[s_tile, t] = kT_chunk^T @ qT (only causal blocks),
    exp fused on ScalarE (scale=1/8) with ONE instruction per (i, span)
    covering both heads via strided 3D APs; diagonal 128x128 block masked by
    a tril multiply on VectorE.
  - AV transposed: out[t, d] += es_chunk^T @ [v | 1] per (head, t-chunk,
    s-chunk): M=128 (full array), N=65; column 64 accumulates the softmax
    denominator, which lands as a per-partition scalar.
  - normalize: DVE reciprocal of col 64 + tensor_scalar multiply into an
    SBUF staging tile; one DMA per head writes out[T, 64h:64h+64] directly.
"""

import numpy as np
import ml_dtypes

import concourse.bass as bass
import concourse.mybir as mybir
from concourse import bacc
from concourse.tile import TileContext
from concourse.bass import ds, ts
from concourse.bass_utils import run_bass_kernel_spmd
from concourse.masks import make_upper_triangular

BF16 = mybir.dt.bfloat16
F32 = mybir.dt.float32

B, T, C, H, HS = 8, 1024, 1024, 16, 64
P = 128
CK = C // P       # 8 contraction chunks
TT = T // P       # 8 t tiles
PAIRS = H // 2    # 8 head pairs
HALF = 512
NV = HS + 1       # v cols + ones col (denominator)

_BUILT = None


def build_nc():
    nc = bacc.Bacc("TRN2", target_bir_lowering=False, debug=False)
    # [p, c, t] : xT[C, T] chunked; partition p, chunk c -> row 128c+p of xT
    xt = nc.dram_tensor("xt", [P, CK, T], BF16, kind="ExternalInput")
    # [proj(q,k), pair, p, c, f] : lhsT chunks, f = 2 heads x 64 stacked
    wqk = nc.dram_tensor("wqk", [2, PAIRS, P, CK, P], BF16, kind="ExternalInput")
    # [p, c, pair, f]
    wv = nc.dram_tensor("wv", [P, CK, PAIRS, P], BF16, kind="ExternalInput")
    # out[T, H*HS] directly -- no host transpose needed
    out = nc.dram_tensor("out", [T, H * HS], F32, kind="ExternalOutput")

    with TileContext(nc) as tc:
        with (
            tc.tile_pool(name="const", bufs=1) as constp,
            tc.tile_pool(name="wpool", bufs=8) as wpool,
            tc.tile_pool(name="qkpool", bufs=8) as qkp,
            tc.tile_pool(name="exppool", bufs=20) as expp,
            tc.tile_pool(name="stgpool", bufs=4) as stgp,
            tc.tile_pool(name="rcppool", bufs=8) as rcpp,
            tc.tile_pool(name="psA", bufs=2, space="PSUM") as psA,
            tc.tile_pool(name="psSc", bufs=2, space="PSUM") as psSc,
            tc.tile_pool(name="psV", bufs=2, space="PSUM") as psV,
        ):
            # ---- PE warmup: keep the array busy from t=0 so HAM goes
            # K=8/8 before the first real projection matmul. ----
            scratch = constp.tile([P, HALF], BF16)
            nc.gpsimd.memset(scratch[:, :], 0.0)
            for wi in range(12):
                pw = psA.tile([P, HALF], F32, tag="ps", name=f"warm{wi}")
                nc.tensor.matmul(pw[:, :], scratch[:, 0:P], scratch[:, :],
                                 start=True, stop=True)

            xt_sb = constp.tile([P, CK, T], BF16)
            # pair-group-major: [p, c, pg, 4*128] so the rhs slice is 2D
            wv_sb = constp.tile([P, CK, 2, 4 * P], BF16)
            # pair-0 Q/K weights first, then xt per chunk: the pair-0
            # projection starts as soon as chunk 0 lands. wv follows xt.
            wq0 = wpool.tile([P, CK, P], BF16, tag="w", name="wq0")
            nc.sync.dma_start(wq0[:, :, :], wqk[0, 0, :, :, :])
            wk0 = wpool.tile([P, CK, P], BF16, tag="w", name="wk0")
            nc.sync.dma_start(wk0[:, :, :], wqk[1, 0, :, :, :])
            for c in range(CK):
                nc.sync.dma_start(xt_sb[:, c, :], xt[:, c, :])
            for c in range(CK):
                nc.sync.dma_start(
                    wv_sb[:, c, :, :],
                    wv[:, c, :, :].rearrange("p (g r) f -> p g (r f)", g=2),
                )
            mask = constp.tile([P, P], BF16)
            make_upper_triangular(nc, mask, val=1.0, diag=True)
            # [s_p, head, s_tile, 64 v cols + 1 ones col]
            v_all = constp.tile([P, H, TT, NV], BF16)
            nc.gpsimd.memset(v_all[:, :, :, HS:NV], 1.0)

            # ---- attention per head pair, software-pipelined ----
            w_next = {}

            def dma_w(pair):
                wq_sb = wpool.tile([P, CK, P], BF16, tag="w", name=f"wq{pair}")
                nc.sync.dma_start(wq_sb[:, :, :], wqk[0, pair, :, :, :])
                wk_sb = wpool.tile([P, CK, P], BF16, tag="w", name=f"wk{pair}")
                nc.sync.dma_start(wk_sb[:, :, :], wqk[1, pair, :, :, :])
                w_next[pair] = (wq_sb, wk_sb)

            def qk_proj(pair):
                qT = qkp.tile([P, T], BF16, tag="qk", name=f"q{pair}")
                kT = qkp.tile([P, T], BF16, tag="qk", name=f"k{pair}")
                for wsb, dst in zip(w_next.pop(pair), (qT, kT)):
                    for g in range(2):
                        pp = psA.tile([P, HALF], F32, tag="ps",
                                      name=f"pp{pair}_{g}")
                        for c in range(CK):
                            nc.tensor.matmul(
                                pp[:, :],
                                wsb[:, c, :],
                                xt_sb[:, c, ds(HALF * g, HALF)],
                                start=(c == 0),
                                stop=(c == CK - 1),
                            )
                        nc.vector.tensor_copy(dst[:, ds(HALF * g, HALF)],
                                              pp[:, :])
                return qT, kT

            # AV for `pair`, software-pipelined one pair behind the scores:
            # running it in the NEXT pair's body breaks the serial
            # exp(p) -> AV(p) -> scores(p+1) -> exp(p+1) dependency cycle
            # that otherwise stretches the pair period to the ACT drain.
            def av_block(pair, es):
                # g-outer / head-inner: head hh's normalize (DVE) overlaps
                # the other head's matmul burst, so the psV slot handoff
                # never idles the PE (matters most for the last pair's tail).
                stgs = [stgp.tile([P, TT, HS], F32, tag="stg",
                                  name=f"stg{pair}_{hh}") for hh in range(2)]
                for g in range(2):
                    for hh in range(2):
                        h = 2 * pair + hh
                        av = psV.tile([P, 4, NV], F32, tag="av",
                                      name=f"av{hh}_{g}")
                        for jj in range(4):
                            j = 4 * g + jj
                            for i in range(j + 1):
                                nc.tensor.matmul(
                                    av[:, jj, :],
                                    es[i][:, ds(T * hh + P * j, P)],
                                    v_all[:, h, i, :],
                                    start=(i == 0),
                                    stop=(i == j),
                                )
                        rcp = rcpp.tile([P, 4, 1], F32, tag="rcp",
                                        name=f"rcp{hh}_{g}")
                        nc.vector.reciprocal(rcp[:, :, :], av[:, :, HS:NV])
                        in0b, in1b = bass.broadcast_tensor_aps(
                            av[:, :, 0:HS], rcp[:, :, :])
                        nc.vector.tensor_tensor(
                            stgs[hh][:, ds(4 * g, 4), :], in0b, in1b,
                            mybir.AluOpType.mult,
                        )
                for hh in range(2):
                    h = 2 * pair + hh
                    nc.sync.dma_start(
                        out.rearrange("(j p) c -> p j c", p=P)[:, :,
                                                              ds(HS * h, HS)],
                        stgs[hh][:, :, :],
                    )

            # ---- prologue: pair-0/1/2 projections interleaved with the v
            # projections. The v work is pure PE filler with no ACT cost, so
            # it runs while the scheduler hoists sc(0..2)'s exp onto the
            # otherwise-idle ScalarE; the pair loop is ACT-bound, so v must
            # NOT sit in a separate phase that starves the ACT. ----
            def v_group(h0, nh, pg, col0, name):
                """v for heads [h0, h0+nh): rhs = wv_sb[:, c, pg, col0:...]"""
                for j in range(TT):
                    pv = psA.tile([P, HALF], F32, tag="ps", name=f"{name}_{j}")
                    for c in range(CK):
                        nc.tensor.matmul(
                            pv[:, 0:HS * nh],
                            xt_sb[:, c, ts(j, P)],
                            wv_sb[:, c, pg, ds(col0, HS * nh)],
                            start=(c == 0),
                            stop=(c == CK - 1),
                        )
                    nc.vector.tensor_copy(
                        v_all[:, ds(h0, nh), j, 0:HS],
                        pv[:, 0:HS * nh].rearrange("p (g d) -> p g d", d=HS),
                    )

            w_next[0] = (wq0, wk0)
            dma_w(1)
            dma_w(2)
            dma_w(3)
            qk_ready = {0: qk_proj(0)}
            qk_ready[1] = qk_proj(1)
            qk_ready[2] = qk_proj(2)
            v_group(0, 8, 0, 0, "v03")
            v_group(8, 8, 1, 0, "v47")

            es_prev = None
            for pair in range(PAIRS):
                qT, kT = qk_ready.pop(pair)

                # scores^T + exp for BOTH heads: two K=64 matmuls in row
                # groups (0,0)/(64,0) run concurrently; ONE merged ACT per
                # (i, span) covers both heads via stride-512 / stride-1024
                # 3D APs. es tile: [128, 2048], head w at col 1024*w.
                # HIGH PRIORITY: the pair loop is ACT(exp)-throughput-bound,
                # so score matmuls must preempt filler (v/qk/AV) the moment
                # their psSc slot frees — otherwise the ScalarE starves.
                es = []
                with tc.high_priority(offset=10 ** 6):
                    for i in range(TT):
                        t0 = P * i
                        e = expp.tile([P, 2 * T], BF16, tag="exp",
                                      name=f"e{i}")
                        es.append(e)
                        e3 = e.rearrange("p (w t) -> p w t", w=2)
                        spans = ([(t0, HALF), (HALF, T)] if t0 < HALF
                                 else [(t0, T)])
                        for a, b in spans:
                            sc = psSc.tile([P, T], F32, tag="sc",
                                           name=f"sc{i}_{a}")
                            for w in range(2):
                                po = HS * w
                                nc.tensor.matmul(
                                    sc[:, ds(HALF * w, b - a)],
                                    kT[ds(po, HS), ds(t0, P)],
                                    qT[ds(po, HS), ds(a, b - a)],
                                )
                            nc.scalar.activation(
                                e3[:, :, a:b],
                                sc.rearrange("p (w t) -> p w t",
                                             w=2)[:, :, 0:b - a],
                                mybir.ActivationFunctionType.Exp,
                                scale=HS ** -0.5,
                            )
                        for w in range(2):
                            nc.vector.tensor_tensor(
                                e3[:, w, t0:t0 + P], e3[:, w, t0:t0 + P],
                                mask[:, :], mybir.AluOpType.mult,
                            )

                # pair p+3's projection runs here, covering the exp drain
                if pair + 3 < PAIRS:
                    qk_ready[pair + 3] = qk_proj(pair + 3)
                if pair + 4 < PAIRS:
                    dma_w(pair + 4)

                # AV transposed (one pair behind): out[t, d] += es^T @ [v|1].
                # M=128 (full), N=65; col 64 = softmax denominator.
                if es_prev is not None:
                    av_block(pair - 1, es_prev)
                es_prev = es
            av_block(PAIRS - 1, es_prev)
    nc.compile()
    return nc


def get_nc():
    global _BUILT
    if _BUILT is None:
        _BUILT = build_nc()
    return _BUILT


def prep_inputs(x, Wq, Wk, Wv):
    """Host-side shard + layout prep. Returns in_maps (one dict per core)."""
    x = np.asarray(x, dtype=np.float32)
    Wq = np.asarray(Wq, dtype=np.float32)
    Wk = np.asarray(Wk, dtype=np.float32)
    Wv = np.asarray(Wv, dtype=np.float32)
    bf = ml_dtypes.bfloat16

    # xT[b]: [C, T] -> [p, c, t] with row 128c+p
    xts = []
    for b in range(B):
        xT = np.ascontiguousarray(x[b].T)          # [C, T]
        xts.append(xT.reshape(CK, P, T).transpose(1, 0, 2).astype(bf))

    def pack_pairs(W):
        # [H, C, hs] -> [pair, C, 128] -> [pair, p, c, f]
        Wp = W.reshape(PAIRS, 2, C, HS).transpose(0, 2, 1, 3).reshape(PAIRS, C, P)
        return Wp.reshape(PAIRS, CK, P, P).transpose(0, 2, 1, 3)  # [pair, p, c, f]

    wq_p = pack_pairs(Wq)
    wk_p = pack_pairs(Wk)
    wqk_host = np.stack([wq_p, wk_p], axis=0).astype(bf)  # [2, pair, p, c, f]
    # wv: [p, c, pair, f]
    wv_host = np.ascontiguousarray(pack_pairs(Wv).transpose(1, 2, 0, 3)).astype(bf)

    return [
        {"xt": np.ascontiguousarray(xts[b]), "wqk": wqk_host, "wv": wv_host}
        for b in range(B)
    ]


def run_on_device(in_maps, **kwargs):
    nc = get_nc()
    return run_bass_kernel_spmd(nc, in_maps, list(range(B)), **kwargs)


def assemble(core_out):
    """Device already writes [T, H*HS]."""
    return np.asarray(core_out)


def kernel(x, Wq, Wk, Wv):
    in_maps = prep_inputs(x, Wq, Wk, Wv)
    res = run_on_device(in_maps)
    return np.stack([assemble(res.results[b]["out"]) for b in range(B)], axis=0)
